# revision 2
# baseline (speedup 1.0000x reference)
"""Trainium2 Bass kernel for nn_CROM_Layer_81140522156285 (moe_routing).

Math restructure (exactly equivalent to the reference, far less work):
  last = x[:, -1, :]
  q    = last @ Wq.T
  qk   = (q @ Wk) / sqrt(D)              # tiny [B, D]
  scores[b, s] = x[b, s, :] . qk[b, :]   # one pass over x  (big, memory-bound)
  attn = softmax(scores)
  ctx  = (attn[b] @ x[b]) @ Wv.T
  out  = ctx @ expert_W[eid].T + expert_b[eid]
  y    = x with last row replaced by LayerNorm(last + out)

Device-side design v2:
  * x bf16, batch-interleaved pack [128, 32, 1024] per core; ALL of x kept
    resident in SBUF (64KB/partition) so DMA streams at full rate without
    buffer-recycling stalls.
  * DMAs issued first thing; escalating sizes so compute starts early.
  * Score pass: one TENSOR_TENSOR_REDUCE per tile on DVE (mult + add-accum)
    if TTR_2X else a balanced V/S split between DVE and ACT.
  * exp per chunk on ACT (+ batch-mask multiply on DVE) -> esc stationary.
  * ctx accumulated on PE: per tile 2 matmuls of N=512 into PSUM [4, 1024];
    z via per-chunk matmul against a ones column.
  * Small last chunk to shrink the end-of-kernel matmul tail.
"""

import numpy as np
import ml_dtypes

import concourse.bass as bass
import concourse.tile as tile
from concourse import bacc, mybir
from concourse.bass_utils import run_bass_kernel_spmd

B = 4
S = 8192
D = 1024
N_CORES = 8
S_CORE = S // N_CORES
P = 128
G = P // B                 # partitions per batch (32)
T = (B * S_CORE) // P      # tiles per core (32)

BF16 = mybir.dt.bfloat16
F32 = mybir.dt.float32

# score path: "ttr" (single fused DVE op / tile) or "vs" (V: DVE-fused stt,
# S: DVE 2x mult + ACT reduce)
SCORE_PATH = "vs"
# chunks of tiles; exp/esc/z and ctx matmuls run per chunk
CHUNKS = [4] * 7 + [3, 1]
# per-chunk DMA grouping (tiles per dma_start), escalating
DMA_SIZES = [2, 2, 4, 4, 4, 4, 4, 4, 4]
# V/S assignment for SCORE_PATH == "vs": per global tile index
VS_NUM_V = 13

_NC = None


def _build_nc():
    nc = bacc.Bacc("TRN2", target_bir_lowering=False, debug=False,
                   num_devices=N_CORES)
    xs_ap = nc.dram_tensor("xs", [P, T, D], BF16, kind="ExternalInput").ap()
    # [:, 0:D] = qk row per batch group; [:, D] = 1.0 (z rhs); [:, D+1] pad;
    # [:, D+2 : D+2+B] = batch-mask columns (1.0 iff p//G == b)
    qkb_ap = nc.dram_tensor("qkb", [P, D + 2 + B], BF16,
                            kind="ExternalInput").ap()
    ctx_ap = nc.dram_tensor("ctx_out", [B, D], F32, kind="ExternalOutput").ap()
    z_ap = nc.dram_tensor("z_out", [B * 4, len(CHUNKS)], F32,
                          kind="ExternalOutput").ap()

    M = mybir.AluOpType.mult
    A = mybir.AluOpType.add

    with tile.TileContext(nc) as tc:
        with (
            tc.tile_pool(name="const", bufs=1) as cpool,
            tc.tile_pool(name="psum", bufs=1, space="PSUM") as psumpool,
        ):
            # ---- all SBUF tiles up front (single-buffered, x fully resident)
            qkb = cpool.tile([P, D + 2 + B], BF16, tag="qkb")
            xtiles = []   # one resident tile per DMA group
            for i, sz in enumerate(DMA_SIZES):
                xg = cpool.tile([P, sz, D], BF16, tag=f"xg{i}")
                xtiles.append(xg)
            # tile index t -> (group, offset)
            xmap = {}
            t0 = 0
            for i, sz in enumerate(DMA_SIZES):
                for k in range(sz):
                    xmap[t0 + k] = (i, k)
                t0 += sz
            dump = cpool.tile([P, D], BF16, tag="dump")
            dump2 = cpool.tile([P, D], BF16, tag="dump2")
            prod = cpool.tile([P, 4, D], BF16, tag="prod")
            # double-buffered per-chunk score/esc tiles
            scs, escds, escs = [], [], []
            for i in range(2):
                sc_i = cpool.tile([P, 4], F32, tag=f"sc{i}")
                escd_i = cpool.tile([P, 4], BF16, tag=f"escd{i}")
                esc_i = cpool.tile([P, 4, B], BF16, tag=f"esc{i}")
                scs.append(sc_i)
                escds.append(escd_i)
                escs.append(esc_i)
            stg = cpool.tile([B, D], F32, tag="stg")
            stgz = cpool.tile([B * 4, len(CHUNKS)], F32, tag="stgz")

            # ---- DMAs first: qkb then x in escalating slices
            nc.sync.dma_start(qkb[:], qkb_ap[:])
            t0 = 0
            for i, sz in enumerate(DMA_SIZES):
                nc.sync.dma_start(xtiles[i][:],
                                  xs_ap[:, t0:t0 + sz, :])
                t0 += sz
            assert t0 == T

            qk = qkb[:, 0:D]
            ones = qkb[:, D:D + 1]
            maskb = qkb[:, D + 2:D + 2 + B]

            ps_ctx = psumpool.tile([B, D], F32, tag="ctx")          # 2 banks
            ps_z = psumpool.tile([B * 4, len(CHUNKS)], F32, tag="z")
            ps_dum = psumpool.tile([1, 2], F32, tag="dum")

            # ---- warmers: trigger ACT exp table load + PE clock during DMA
            nc.scalar.activation(escds[0][:], qkb[:, D:D + 4],
                                 mybir.ActivationFunctionType.Exp)
            for i in range(4):
                nc.tensor.matmul(ps_dum[:, 0:1], qkb[:, D + i:D + i + 1],
                                 qkb[:, D:D + 1], start=True, stop=True)

            n_v = VS_NUM_V

            # ---- main loop
            t0 = 0
            for ci, W in enumerate(CHUNKS):
                sc = scs[ci % 2]
                escd = escds[ci % 2]
                esc = escs[ci % 2]
                for w in range(W):
                    t = t0 + w
                    gi, gk = xmap[t]
                    xt = xtiles[gi][:, gk, :]
                    if SCORE_PATH == "ttr":
                        nc.vector.tensor_tensor_reduce(
                            out=(dump if t % 2 == 0 else dump2)[:],
                            in0=xt, in1=qk, scale=1.0,
                            scalar=0.0, op0=M, op1=A,
                            accum_out=sc[:, w:w + 1])
                    else:
                        # vs split: first VS_NUM_V global tiles -> V (fused),
                        # interleaved with S tiles
                        is_v = (t * VS_NUM_V) // T != ((t + 1) * VS_NUM_V) // T
                        if is_v:
                            nc.vector.scalar_tensor_tensor(
                                out=dump[:], in0=xt, scalar=1.0, in1=qk,
                                op0=M, op1=M, accum_out=sc[:, w:w + 1])
                        else:
                            pj = t % 4
                            nc.vector.tensor_tensor(
                                out=prod[:, pj, :], in0=xt, in1=qk, op=M)
                            nc.scalar.activation(
                                dump2[:], prod[:, pj, :],
                                mybir.ActivationFunctionType.Copy,
                                accum_out=sc[:, w:w + 1])
                    # PE warmth: tiny dummy matmul pinned to this score col
                    if t % 2 == 0:
                        nc.tensor.matmul(ps_dum[:, 1:2], sc[:, w:w + 1],
                                         sc[:, w:w + 1], start=True, stop=True)

                # exp + batch-mask -> esc [P, W, B]
                nc.scalar.activation(escd[:, 0:W], sc[:, 0:W],
                                     mybir.ActivationFunctionType.Exp)
                nc.vector.tensor_tensor(
                    out=esc[:, 0:W, :],
                    in0=escd[:, 0:W].unsqueeze(2).broadcast_to([P, W, B]),
                    in1=maskb.unsqueeze(1).broadcast_to([P, W, B]),
                    op=M)

                for w in range(W):
                    t = t0 + w
                    gi, gk = xmap[t]
                    xt = xtiles[gi]
                    st, sp = (t == 0), (t == T - 1)
                    nc.tensor.matmul(ps_ctx[:, 0:512], esc[:, w, :],
                                     xt[:, gk, 0:512], start=st, stop=sp)
                    nc.tensor.matmul(ps_ctx[:, 512:1024], esc[:, w, :],
                                     xt[:, gk, 512:1024], start=st, stop=sp)
                # z for this chunk: esc flat [P, W*B] @ ones -> rows w*B+b
                nc.tensor.matmul(ps_z[0:B * W, ci:ci + 1],
                                 esc[:, 0:W, :].rearrange("p a b -> p (a b)"),
                                 ones, start=True, stop=True)
                t0 += W

            nc.vector.tensor_copy(stgz[:], ps_z[:])
            nc.sync.dma_start(z_ap[:], stgz[:])
            # split the ctx evacuation across DVE and ACT to halve the tail
            nc.vector.tensor_copy(stg[:, 0:512], ps_ctx[:, 0:512])
            nc.scalar.activation(stg[:, 512:1024], ps_ctx[:, 512:1024],
                                 mybir.ActivationFunctionType.Copy)
            nc.sync.dma_start(ctx_ap[:], stg[:])

    nc.compile()
    return nc


def _get_nc():
    global _NC
    if _NC is None:
        _NC = _build_nc()
    return _NC


def kernel(x_emb, Wq, Wk, Wv, expert_W, expert_b, ln_gamma, ln_beta,
           expert_id, _spmd_kwargs=None):
    x = np.ascontiguousarray(np.asarray(x_emb, dtype=np.float32))
    Wq = np.asarray(Wq, dtype=np.float32)
    Wk = np.asarray(Wk, dtype=np.float32)
    Wv = np.asarray(Wv, dtype=np.float32)
    expert_b = np.asarray(expert_b, dtype=np.float32)
    ln_gamma = np.asarray(ln_gamma, dtype=np.float32)
    ln_beta = np.asarray(ln_beta, dtype=np.float32)
    eid = int(np.asarray(expert_id))

    last = x[:, -1, :]                                   # [B, D]
    q = last @ Wq.T                                      # [B, D]
    qk = (q @ Wk) * np.float32(1.0 / np.sqrt(D))         # [B, D]

    qkb = np.zeros((P, D + 2 + B), dtype=ml_dtypes.bfloat16)
    qkb[:, 0:D] = np.repeat(qk, G, axis=0).astype(ml_dtypes.bfloat16)
    qkb[:, D] = ml_dtypes.bfloat16(1.0)
    for b in range(B):
        qkb[b * G:(b + 1) * G, D + 2 + b] = ml_dtypes.bfloat16(1.0)

    in_maps = []
    for c in range(N_CORES):
        shard = x[:, c * S_CORE:(c + 1) * S_CORE, :]     # [B, S_CORE, D]
        xs = np.ascontiguousarray(
            shard.reshape(P, T, D).astype(ml_dtypes.bfloat16))
        in_maps.append({"xs": xs, "qkb": qkb})

    res = run_bass_kernel_spmd(_get_nc(), in_maps, core_ids=list(range(N_CORES)),
                               **(_spmd_kwargs or {}))
    ctx_un = np.zeros((B, D), dtype=np.float32)
    z = np.zeros((B, 1), dtype=np.float32)
    for c in range(N_CORES):
        ctx_un += res.results[c]["ctx_out"]
        zo = res.results[c]["z_out"]                     # [16, nchunk]
        for ci, W in enumerate(CHUNKS):
            blk = zo[0:B * W, ci].reshape(W, B)          # row = w*B + b
            z[:, 0] += blk.sum(axis=0)

    ctx = ctx_un / z                                     # [B, D] attn @ x
    context = ctx @ Wv.T                                 # [B, D]
    We = np.asarray(expert_W[eid], dtype=np.float32)     # [D, D]
    out = context @ We.T + expert_b[eid]                 # [B, D]
    resid = last + out
    mu = resid.mean(axis=-1, keepdims=True, dtype=np.float32)
    diff = resid - mu
    var = np.mean(diff * diff, axis=-1, keepdims=True, dtype=np.float32)
    new_focus = diff / np.sqrt(var + np.float32(1e-5)) * ln_gamma + ln_beta

    y = x.copy()
    y[:, -1, :] = new_focus
    return y


if __name__ == "__main__":
    rng = np.random.default_rng(0)
    xs = {
        "x_emb": rng.standard_normal((B, S, D), dtype=np.float32),
        "Wq": rng.standard_normal((D, D), dtype=np.float32) * 0.02,
        "Wk": rng.standard_normal((D, D), dtype=np.float32) * 0.02,
        "Wv": rng.standard_normal((D, D), dtype=np.float32) * 0.02,
        "expert_W": rng.standard_normal((128, D, D), dtype=np.float32) * 0.02,
        "expert_b": rng.standard_normal((128, D), dtype=np.float32) * 0.02,
        "ln_gamma": np.ones(D, dtype=np.float32),
        "ln_beta": np.zeros(D, dtype=np.float32),
        "expert_id": 7,
    }
    y = kernel(**xs)
    print(y.shape, y.dtype)


# revision 3
# speedup vs baseline: 1.0356x; 1.0356x over previous
"""Trainium2 Bass kernel for nn_CROM_Layer_81140522156285 (moe_routing).

Math restructure (exactly equivalent to the reference, far less work):
  last = x[:, -1, :]
  q    = last @ Wq.T
  qk   = (q @ Wk) / sqrt(D)              # tiny [B, D]
  scores[b, s] = x[b, s, :] . qk[b, :]   # one pass over x  (big, memory-bound)
  attn = softmax(scores)
  ctx  = (attn[b] @ x[b]) @ Wv.T
  out  = ctx @ expert_W[eid].T + expert_b[eid]
  y    = x with last row replaced by LayerNorm(last + out)

Device-side design v2:
  * x bf16, batch-interleaved pack [128, 32, 1024] per core; ALL of x kept
    resident in SBUF (64KB/partition) so DMA streams at full rate without
    buffer-recycling stalls.
  * DMAs issued first thing; escalating sizes so compute starts early.
  * Score pass: balanced V/S split — 13 'V' tiles use the DVE fused
    scalar_tensor_tensor (mult+accum, 1x, ~1.3us), 19 'S' tiles use a DVE
    2x multiply (~0.7us) + ScalarE copy-accum reduce (~1.46us), so both
    engines run ~30us dense.
  * exp per chunk on ACT (+ batch-mask multiply on DVE) -> esc stationary.
  * ctx accumulated on PE: per tile 2 matmuls of N=512 into PSUM [4, 1024];
    z via per-chunk matmul against a ones column.
  * Small last chunk to shrink the end-of-kernel matmul tail.
"""

import numpy as np
import ml_dtypes

import concourse.bass as bass
import concourse.tile as tile
from concourse import bacc, mybir
from concourse.bass_utils import run_bass_kernel_spmd

B = 4
S = 8192
D = 1024
N_CORES = 8
S_CORE = S // N_CORES
P = 128
G = P // B                 # partitions per batch (32)
T = (B * S_CORE) // P      # tiles per core (32)

BF16 = mybir.dt.bfloat16
F32 = mybir.dt.float32

# score path: "ttr" (single fused DVE op / tile) or "vs" (V: DVE-fused stt,
# S: DVE 2x mult + ACT reduce)
SCORE_PATH = "vs"
# chunks of tiles; exp/esc/z and ctx matmuls run per chunk
CHUNKS = [4] * 7 + [3, 1]
# per-chunk DMA grouping (tiles per dma_start), escalating
DMA_SIZES = [2, 2, 4, 4, 4, 4, 4, 4, 4]
# V/S assignment for SCORE_PATH == "vs": per global tile index
VS_NUM_V = 13

_NC = None


def _build_nc():
    nc = bacc.Bacc("TRN2", target_bir_lowering=False, debug=False,
                   num_devices=N_CORES)
    xs_ap = nc.dram_tensor("xs", [P, T, D], BF16, kind="ExternalInput").ap()
    # [:, 0:D] = qk row per batch group; [:, D] = 1.0 (z rhs); [:, D+1] pad;
    # [:, D+2 : D+2+B] = batch-mask columns (1.0 iff p//G == b)
    qkb_ap = nc.dram_tensor("qkb", [P, D + 2 + B], BF16,
                            kind="ExternalInput").ap()
    ctx_ap = nc.dram_tensor("ctx_out", [B, D], F32, kind="ExternalOutput").ap()
    z_ap = nc.dram_tensor("z_out", [B * 4, len(CHUNKS)], F32,
                          kind="ExternalOutput").ap()

    M = mybir.AluOpType.mult
    A = mybir.AluOpType.add

    with tile.TileContext(nc) as tc:
        with (
            tc.tile_pool(name="const", bufs=1) as cpool,
            tc.tile_pool(name="psum", bufs=1, space="PSUM") as psumpool,
        ):
            # ---- all SBUF tiles up front (single-buffered, x fully resident)
            qkb = cpool.tile([P, D + 2 + B], BF16, tag="qkb")
            xtiles = []   # one resident tile per DMA group
            for i, sz in enumerate(DMA_SIZES):
                xg = cpool.tile([P, sz, D], BF16, tag=f"xg{i}")
                xtiles.append(xg)
            # tile index t -> (group, offset)
            xmap = {}
            t0 = 0
            for i, sz in enumerate(DMA_SIZES):
                for k in range(sz):
                    xmap[t0 + k] = (i, k)
                t0 += sz
            dump = cpool.tile([P, D], BF16, tag="dump")
            dump2 = cpool.tile([P, D], BF16, tag="dump2")
            prod = cpool.tile([P, 4, D], BF16, tag="prod")
            # double-buffered per-chunk score/esc tiles
            scs, escds, escs = [], [], []
            for i in range(2):
                sc_i = cpool.tile([P, 4], F32, tag=f"sc{i}")
                escd_i = cpool.tile([P, 4], BF16, tag=f"escd{i}")
                esc_i = cpool.tile([P, 4, B], BF16, tag=f"esc{i}")
                scs.append(sc_i)
                escds.append(escd_i)
                escs.append(esc_i)
            stg = cpool.tile([B, D], F32, tag="stg")
            stgz = cpool.tile([B * 4, len(CHUNKS)], F32, tag="stgz")

            # ---- DMAs first: qkb then x in escalating slices
            nc.sync.dma_start(qkb[:], qkb_ap[:])
            t0 = 0
            for i, sz in enumerate(DMA_SIZES):
                nc.sync.dma_start(xtiles[i][:],
                                  xs_ap[:, t0:t0 + sz, :])
                t0 += sz
            assert t0 == T

            qk = qkb[:, 0:D]
            ones = qkb[:, D:D + 1]
            maskb = qkb[:, D + 2:D + 2 + B]

            ps_ctx = psumpool.tile([B, D], F32, tag="ctx")          # 2 banks
            ps_z = psumpool.tile([B * 4, len(CHUNKS)], F32, tag="z")
            ps_dum = psumpool.tile([1, 2], F32, tag="dum")

            # ---- warmers: trigger ACT exp table load + PE clock during DMA
            nc.scalar.activation(escds[0][:], qkb[:, D:D + 4],
                                 mybir.ActivationFunctionType.Exp)
            for i in range(4):
                nc.tensor.matmul(ps_dum[:, 0:1], qkb[:, D + i:D + i + 1],
                                 qkb[:, D:D + 1], start=True, stop=True)

            n_v = VS_NUM_V

            # ---- main loop
            t0 = 0
            for ci, W in enumerate(CHUNKS):
                sc = scs[ci % 2]
                escd = escds[ci % 2]
                esc = escs[ci % 2]
                for w in range(W):
                    t = t0 + w
                    gi, gk = xmap[t]
                    xt = xtiles[gi][:, gk, :]
                    if SCORE_PATH == "ttr":
                        nc.vector.tensor_tensor_reduce(
                            out=(dump if t % 2 == 0 else dump2)[:],
                            in0=xt, in1=qk, scale=1.0,
                            scalar=0.0, op0=M, op1=A,
                            accum_out=sc[:, w:w + 1])
                    else:
                        # vs split: first VS_NUM_V global tiles -> V (fused),
                        # interleaved with S tiles
                        is_v = (t * VS_NUM_V) // T != ((t + 1) * VS_NUM_V) // T
                        if is_v:
                            nc.vector.scalar_tensor_tensor(
                                out=dump[:], in0=xt, scalar=1.0, in1=qk,
                                op0=M, op1=M, accum_out=sc[:, w:w + 1])
                        else:
                            pj = t % 4
                            nc.vector.tensor_tensor(
                                out=prod[:, pj, :], in0=xt, in1=qk, op=M)
                            nc.scalar.activation(
                                dump2[:], prod[:, pj, :],
                                mybir.ActivationFunctionType.Copy,
                                accum_out=sc[:, w:w + 1])
                    # PE warmth: tiny dummy matmul pinned to this score col
                    if t % 2 == 0:
                        nc.tensor.matmul(ps_dum[:, 1:2], sc[:, w:w + 1],
                                         sc[:, w:w + 1], start=True, stop=True)

                # exp + batch-mask -> esc [P, W, B]
                nc.scalar.activation(escd[:, 0:W], sc[:, 0:W],
                                     mybir.ActivationFunctionType.Exp)
                nc.vector.tensor_tensor(
                    out=esc[:, 0:W, :],
                    in0=escd[:, 0:W].unsqueeze(2).broadcast_to([P, W, B]),
                    in1=maskb.unsqueeze(1).broadcast_to([P, W, B]),
                    op=M)

                for w in range(W):
                    t = t0 + w
                    gi, gk = xmap[t]
                    xt = xtiles[gi]
                    st, sp = (t == 0), (t == T - 1)
                    nc.tensor.matmul(ps_ctx[:, 0:512], esc[:, w, :],
                                     xt[:, gk, 0:512], start=st, stop=sp)
                    nc.tensor.matmul(ps_ctx[:, 512:1024], esc[:, w, :],
                                     xt[:, gk, 512:1024], start=st, stop=sp)
                # z for this chunk: esc flat [P, W*B] @ ones -> rows w*B+b
                nc.tensor.matmul(ps_z[0:B * W, ci:ci + 1],
                                 esc[:, 0:W, :].rearrange("p a b -> p (a b)"),
                                 ones, start=True, stop=True)
                t0 += W

            nc.vector.tensor_copy(stgz[:], ps_z[:])
            nc.sync.dma_start(z_ap[:], stgz[:])
            # split the ctx evacuation across DVE and ACT to halve the tail
            nc.vector.tensor_copy(stg[:, 0:512], ps_ctx[:, 0:512])
            nc.scalar.activation(stg[:, 512:1024], ps_ctx[:, 512:1024],
                                 mybir.ActivationFunctionType.Copy)
            nc.sync.dma_start(ctx_ap[:], stg[:])

    nc.compile()
    return nc


def _get_nc():
    global _NC
    if _NC is None:
        _NC = _build_nc()
    return _NC


def kernel(x_emb, Wq, Wk, Wv, expert_W, expert_b, ln_gamma, ln_beta,
           expert_id, _spmd_kwargs=None):
    x = np.ascontiguousarray(np.asarray(x_emb, dtype=np.float32))
    Wq = np.asarray(Wq, dtype=np.float32)
    Wk = np.asarray(Wk, dtype=np.float32)
    Wv = np.asarray(Wv, dtype=np.float32)
    expert_b = np.asarray(expert_b, dtype=np.float32)
    ln_gamma = np.asarray(ln_gamma, dtype=np.float32)
    ln_beta = np.asarray(ln_beta, dtype=np.float32)
    eid = int(np.asarray(expert_id))

    last = x[:, -1, :]                                   # [B, D]
    q = last @ Wq.T                                      # [B, D]
    qk = (q @ Wk) * np.float32(1.0 / np.sqrt(D))         # [B, D]

    qkb = np.zeros((P, D + 2 + B), dtype=ml_dtypes.bfloat16)
    qkb[:, 0:D] = np.repeat(qk, G, axis=0).astype(ml_dtypes.bfloat16)
    qkb[:, D] = ml_dtypes.bfloat16(1.0)
    for b in range(B):
        qkb[b * G:(b + 1) * G, D + 2 + b] = ml_dtypes.bfloat16(1.0)

    in_maps = []
    for c in range(N_CORES):
        shard = x[:, c * S_CORE:(c + 1) * S_CORE, :]     # [B, S_CORE, D]
        xs = np.ascontiguousarray(
            shard.reshape(P, T, D).astype(ml_dtypes.bfloat16))
        in_maps.append({"xs": xs, "qkb": qkb})

    res = run_bass_kernel_spmd(_get_nc(), in_maps, core_ids=list(range(N_CORES)),
                               **(_spmd_kwargs or {}))
    ctx_un = np.zeros((B, D), dtype=np.float32)
    z = np.zeros((B, 1), dtype=np.float32)
    for c in range(N_CORES):
        ctx_un += res.results[c]["ctx_out"]
        zo = res.results[c]["z_out"]                     # [16, nchunk]
        for ci, W in enumerate(CHUNKS):
            blk = zo[0:B * W, ci].reshape(W, B)          # row = w*B + b
            z[:, 0] += blk.sum(axis=0)

    ctx = ctx_un / z                                     # [B, D] attn @ x
    context = ctx @ Wv.T                                 # [B, D]
    We = np.asarray(expert_W[eid], dtype=np.float32)     # [D, D]
    out = context @ We.T + expert_b[eid]                 # [B, D]
    resid = last + out
    mu = resid.mean(axis=-1, keepdims=True, dtype=np.float32)
    diff = resid - mu
    var = np.mean(diff * diff, axis=-1, keepdims=True, dtype=np.float32)
    new_focus = diff / np.sqrt(var + np.float32(1e-5)) * ln_gamma + ln_beta

    y = x.copy()
    y[:, -1, :] = new_focus
    return y


if __name__ == "__main__":
    rng = np.random.default_rng(0)
    xs = {
        "x_emb": rng.standard_normal((B, S, D), dtype=np.float32),
        "Wq": rng.standard_normal((D, D), dtype=np.float32) * 0.02,
        "Wk": rng.standard_normal((D, D), dtype=np.float32) * 0.02,
        "Wv": rng.standard_normal((D, D), dtype=np.float32) * 0.02,
        "expert_W": rng.standard_normal((128, D, D), dtype=np.float32) * 0.02,
        "expert_b": rng.standard_normal((128, D), dtype=np.float32) * 0.02,
        "ln_gamma": np.ones(D, dtype=np.float32),
        "ln_beta": np.zeros(D, dtype=np.float32),
        "expert_id": 7,
    }
    y = kernel(**xs)
    print(y.shape, y.dtype)


# revision 4
# speedup vs baseline: 1.0841x; 1.0469x over previous
"""Trainium2 Bass kernel for nn_CROM_Layer_81140522156285 (moe_routing).

Math restructure (exactly equivalent to the reference, far less work):
  last = x[:, -1, :]
  q    = last @ Wq.T
  qk   = (q @ Wk) / sqrt(D)              # tiny [B, D]
  scores[b, s] = x[b, s, :] . qk[b, :]   # one pass over x  (big, memory-bound)
  attn = softmax(scores)
  ctx  = (attn[b] @ x[b]) @ Wv.T
  out  = ctx @ expert_W[eid].T + expert_b[eid]
  y    = x with last row replaced by LayerNorm(last + out)

Device-side design v2:
  * x bf16, batch-interleaved pack [128, 32, 1024] per core; ALL of x kept
    resident in SBUF (64KB/partition) so DMA streams at full rate without
    buffer-recycling stalls.
  * DMAs issued first thing; escalating sizes so compute starts early.
  * Score pass: one TENSOR_TENSOR_REDUCE per tile on DVE (mult + add-accum)
    if TTR_2X else a balanced V/S split between DVE and ACT.
  * exp per chunk on ACT (+ batch-mask multiply on DVE) -> esc stationary.
  * ctx accumulated on PE: per tile 2 matmuls of N=512 into PSUM [4, 1024];
    z via per-chunk matmul against a ones column.
  * Small last chunk to shrink the end-of-kernel matmul tail.
"""

import numpy as np
import ml_dtypes

import concourse.bass as bass
import concourse.tile as tile
from concourse import bacc, mybir
from concourse.bass_utils import run_bass_kernel_spmd

B = 4
S = 8192
D = 1024
N_CORES = 8
S_CORE = S // N_CORES
P = 128
G = P // B                 # partitions per batch (32)
T = (B * S_CORE) // P      # tiles per core (32)

BF16 = mybir.dt.bfloat16
F32 = mybir.dt.float32
FP8 = mybir.dt.float8e4

# tiles whose score dot-product runs on the TensorEngine from an fp8
# d-layout copy (x-as-stationary, qk as moving; out = [128 pos, 4] PSUM)
PE_TILES = (12, 16, 20, 24, 26, 28, 30)
QK8_SCALE = 64.0

# score path: "ttr" (single fused DVE op / tile) or "vs" (V: DVE-fused stt,
# S: DVE 2x mult + ACT reduce)
SCORE_PATH = "vs"
# chunks of tiles; exp/esc/z and ctx matmuls run per chunk
CHUNKS = [4] * 7 + [3, 1]
# per-chunk DMA grouping (tiles per dma_start), escalating
DMA_SIZES = [2, 2, 4, 4, 4, 4, 4, 4, 4]
# V/S assignment for SCORE_PATH == "vs": per global tile index
VS_NUM_V = 10


def _tile_kind(t):
    """'P' (PE score), 'V' (DVE fused), or 'S' (DVE mult + ACT reduce)."""
    if t in PE_TILES:
        return "P"
    idx = sum(1 for u in range(t) if u not in PE_TILES)
    n = T - len(PE_TILES)
    if (idx * VS_NUM_V) // n != ((idx + 1) * VS_NUM_V) // n:
        return "V"
    return "S"

_NC = None


def _build_nc():
    nc = bacc.Bacc("TRN2", target_bir_lowering=False, debug=False,
                   num_devices=N_CORES)
    xs_ap = nc.dram_tensor("xs", [P, T, D], BF16, kind="ExternalInput").ap()
    # [:, 0:D] = qk row per batch group; [:, D] = 1.0 (z rhs); [:, D+1] pad;
    # [:, D+2 : D+2+B] = batch-mask columns (1.0 iff p//G == b);
    # [:, D+2+B : D+2+B+4] = zeros (sc init for PE-scored tiles)
    qkb_ap = nc.dram_tensor("qkb", [P, D + 2 + B + 4], BF16,
                            kind="ExternalInput").ap()
    # d-layout fp8 stationary blocks: xd8[p, j, c, q] = x_tile[PE_TILES[j]][q, 128c+p]
    xd8_ap = nc.dram_tensor("xd8", [P, len(PE_TILES), 8, P], FP8,
                            kind="ExternalInput").ap()
    # qk8[p, c, b] = qk[b, 128c+p] * QK8_SCALE
    qk8_ap = nc.dram_tensor("qk8", [P, 8, B], FP8, kind="ExternalInput").ap()
    ctx_ap = nc.dram_tensor("ctx_out", [B, D], F32, kind="ExternalOutput").ap()
    z_ap = nc.dram_tensor("z_out", [B * 4, len(CHUNKS)], F32,
                          kind="ExternalOutput").ap()

    M = mybir.AluOpType.mult
    A = mybir.AluOpType.add

    with tile.TileContext(nc) as tc:
        with (
            tc.tile_pool(name="const", bufs=1) as cpool,
            tc.tile_pool(name="psum", bufs=1, space="PSUM") as psumpool,
        ):
            # ---- all SBUF tiles up front (single-buffered, x fully resident)
            qkb = cpool.tile([P, D + 2 + B + 4], BF16, tag="qkb")
            xd8 = cpool.tile([P, len(PE_TILES), 8, P], FP8, tag="xd8")
            qk8 = cpool.tile([P, 8, B], FP8, tag="qk8")
            escq = cpool.tile([P, B], BF16, tag="escq")
            xtiles = []   # one resident tile per DMA group
            for i, sz in enumerate(DMA_SIZES):
                xg = cpool.tile([P, sz, D], BF16, tag=f"xg{i}")
                xtiles.append(xg)
            # tile index t -> (group, offset)
            xmap = {}
            t0 = 0
            for i, sz in enumerate(DMA_SIZES):
                for k in range(sz):
                    xmap[t0 + k] = (i, k)
                t0 += sz
            dump = cpool.tile([P, D], BF16, tag="dump")
            dump2 = cpool.tile([P, D], BF16, tag="dump2")
            prod = cpool.tile([P, 4, D], BF16, tag="prod")
            # double-buffered per-chunk score/esc tiles
            # esc per chunk (no recycling): the PE may lag several chunks
            # behind the elementwise pipeline; recycled esc buffers would
            # stall DVE on ctx-matmul WAR hazards
            scs, escds, escs = [], [], []
            for i in range(2):
                sc_i = cpool.tile([P, 4], F32, tag=f"sc{i}")
                escd_i = cpool.tile([P, 4], BF16, tag=f"escd{i}")
                scs.append(sc_i)
                escds.append(escd_i)
            for i in range(len(CHUNKS)):
                esc_i = cpool.tile([P, 4, B], BF16, tag=f"esc{i}")
                escs.append(esc_i)
            stg = cpool.tile([B, D], F32, tag="stg")
            stgz = cpool.tile([B * 4, len(CHUNKS)], F32, tag="stgz")

            # ---- DMAs first: qkb/qk8 then x in escalating slices; the fp8
            # d-blocks ride after the first two x groups (needed from ~t=4)
            nc.sync.dma_start(qkb[:], qkb_ap[:])
            nc.sync.dma_start(qk8[:], qk8_ap[:])
            t0 = 0
            for i, sz in enumerate(DMA_SIZES):
                nc.sync.dma_start(xtiles[i][:],
                                  xs_ap[:, t0:t0 + sz, :])
                t0 += sz
                if i == 3:
                    nc.sync.dma_start(xd8[:], xd8_ap[:])
            assert t0 == T

            qk = qkb[:, 0:D]
            ones = qkb[:, D:D + 1]
            maskb = qkb[:, D + 2:D + 2 + B]

            ps_ctx = psumpool.tile([B, D], F32, tag="ctx")          # 2 banks
            ps_z = psumpool.tile([B * 4, len(CHUNKS)], F32, tag="z")
            ps_dum = psumpool.tile([1, 2], F32, tag="dum")
            ps_sc = psumpool.tile([P, len(PE_TILES), B], F32, tag="psc")

            # ---- warmers: trigger ACT exp table load + PE clock during DMA
            nc.scalar.activation(escds[0][:], qkb[:, D:D + 4],
                                 mybir.ActivationFunctionType.Exp)
            for i in range(4):
                nc.tensor.matmul(ps_dum[:, 0:1], qkb[:, D + i:D + i + 1],
                                 qkb[:, D:D + 1], start=True, stop=True)
            # zero-init sc buffers: PE-scored tiles never write their sc
            # column, and exp(garbage) -> Inf * mask(0) -> NaN otherwise
            zc = qkb[:, D + 2 + B:D + 2 + B + 4]
            nc.vector.tensor_copy(scs[0][:], zc)
            nc.vector.tensor_copy(scs[1][:], zc)

            # all PE-tile score matmuls up front, off the chunk chain: the
            # PE is idle while DVE/ACT ramp, and each group accumulates into
            # its own [P, B] slice of one PSUM bank
            for j in range(len(PE_TILES)):
                for c in range(8):
                    nc.tensor.matmul(ps_sc[:, j, :], xd8[:, j, c, :],
                                     qk8[:, c, :],
                                     start=(c == 0), stop=(c == 7))

            n_v = VS_NUM_V

            # ---- main loop
            t0 = 0
            for ci, W in enumerate(CHUNKS):
                sc = scs[ci % 2]
                escd = escds[ci % 2]
                esc = escs[ci]
                for w in range(W):
                    t = t0 + w
                    gi, gk = xmap[t]
                    xt = xtiles[gi][:, gk, :]
                    if SCORE_PATH == "ttr":
                        nc.vector.tensor_tensor_reduce(
                            out=(dump if t % 2 == 0 else dump2)[:],
                            in0=xt, in1=qk, scale=1.0,
                            scalar=0.0, op0=M, op1=A,
                            accum_out=sc[:, w:w + 1])
                    else:
                        kind = _tile_kind(t)
                        if kind == "P":
                            pass  # scores already accumulated up front
                        elif kind == "V":
                            nc.vector.scalar_tensor_tensor(
                                out=dump[:], in0=xt, scalar=1.0, in1=qk,
                                op0=M, op1=M, accum_out=sc[:, w:w + 1])
                        else:
                            pj = t % 4
                            nc.vector.tensor_tensor(
                                out=prod[:, pj, :], in0=xt, in1=qk, op=M)
                            nc.scalar.activation(
                                dump2[:], prod[:, pj, :],
                                mybir.ActivationFunctionType.Copy,
                                accum_out=sc[:, w:w + 1])
                    # PE warmth: tiny dummy matmul pinned to this score col
                    if t % 2 == 0 and t not in PE_TILES:
                        nc.tensor.matmul(ps_dum[:, 1:2], sc[:, w:w + 1],
                                         sc[:, w:w + 1], start=True, stop=True)

                # exp + batch-mask -> esc [P, W, B]
                nc.scalar.activation(escd[:, 0:W], sc[:, 0:W],
                                     mybir.ActivationFunctionType.Exp)
                nc.vector.tensor_tensor(
                    out=esc[:, 0:W, :],
                    in0=escd[:, 0:W].unsqueeze(2).broadcast_to([P, W, B]),
                    in1=maskb.unsqueeze(1).broadcast_to([P, W, B]),
                    op=M)
                # PE-scored tiles: exp straight from PSUM (undo QK8_SCALE),
                # then mask; overwrites this tile's esc slice
                for w in range(W):
                    t = t0 + w
                    if t in PE_TILES:
                        j = PE_TILES.index(t)
                        nc.scalar.activation(
                            escq[:], ps_sc[:, j, :],
                            mybir.ActivationFunctionType.Exp,
                            scale=1.0 / QK8_SCALE)
                        nc.vector.tensor_tensor(
                            out=esc[:, w, :], in0=escq[:], in1=maskb, op=M)

                for w in range(W):
                    t = t0 + w
                    gi, gk = xmap[t]
                    xt = xtiles[gi]
                    st, sp = (t == 0), (t == T - 1)
                    nc.tensor.matmul(ps_ctx[:, 0:512], esc[:, w, :],
                                     xt[:, gk, 0:512], start=st, stop=sp)
                    nc.tensor.matmul(ps_ctx[:, 512:1024], esc[:, w, :],
                                     xt[:, gk, 512:1024], start=st, stop=sp)
                # z for this chunk: esc flat [P, W*B] @ ones -> rows w*B+b
                nc.tensor.matmul(ps_z[0:B * W, ci:ci + 1],
                                 esc[:, 0:W, :].rearrange("p a b -> p (a b)"),
                                 ones, start=True, stop=True)
                t0 += W

            nc.vector.tensor_copy(stgz[:], ps_z[:])
            nc.sync.dma_start(z_ap[:], stgz[:])
            # split the ctx evacuation across DVE and ACT to halve the tail
            nc.vector.tensor_copy(stg[:, 0:512], ps_ctx[:, 0:512])
            nc.scalar.activation(stg[:, 512:1024], ps_ctx[:, 512:1024],
                                 mybir.ActivationFunctionType.Copy)
            nc.sync.dma_start(ctx_ap[:], stg[:])

    nc.compile()
    return nc


def _get_nc():
    global _NC
    if _NC is None:
        _NC = _build_nc()
    return _NC


def kernel(x_emb, Wq, Wk, Wv, expert_W, expert_b, ln_gamma, ln_beta,
           expert_id, _spmd_kwargs=None):
    x = np.ascontiguousarray(np.asarray(x_emb, dtype=np.float32))
    Wq = np.asarray(Wq, dtype=np.float32)
    Wk = np.asarray(Wk, dtype=np.float32)
    Wv = np.asarray(Wv, dtype=np.float32)
    expert_b = np.asarray(expert_b, dtype=np.float32)
    ln_gamma = np.asarray(ln_gamma, dtype=np.float32)
    ln_beta = np.asarray(ln_beta, dtype=np.float32)
    eid = int(np.asarray(expert_id))

    last = x[:, -1, :]                                   # [B, D]
    q = last @ Wq.T                                      # [B, D]
    qk = (q @ Wk) * np.float32(1.0 / np.sqrt(D))         # [B, D]

    qkb = np.zeros((P, D + 2 + B + 4), dtype=ml_dtypes.bfloat16)
    qkb[:, 0:D] = np.repeat(qk, G, axis=0).astype(ml_dtypes.bfloat16)
    qkb[:, D] = ml_dtypes.bfloat16(1.0)
    for b in range(B):
        qkb[b * G:(b + 1) * G, D + 2 + b] = ml_dtypes.bfloat16(1.0)

    f8 = ml_dtypes.float8_e4m3fn
    # qk8[p, c, b] = qk[b, 128c+p] * QK8_SCALE
    qk8 = np.ascontiguousarray(
        np.transpose((qk.T * np.float32(QK8_SCALE)).reshape(8, P, B),
                     (1, 0, 2))).astype(f8)

    in_maps = []
    for c in range(N_CORES):
        shard = x[:, c * S_CORE:(c + 1) * S_CORE, :]     # [B, S_CORE, D]
        xs32 = shard.reshape(P, T, D)
        xs = np.ascontiguousarray(xs32.astype(ml_dtypes.bfloat16))
        # xd8[p, j, c, q] = x_tile[PE_TILES[j]][q, 128c+p]
        xd8 = np.ascontiguousarray(
            np.transpose(
                xs32[:, list(PE_TILES), :].reshape(P, len(PE_TILES), 8, P),
                (3, 1, 2, 0))).astype(f8)
        in_maps.append({"xs": xs, "qkb": qkb, "xd8": xd8, "qk8": qk8})

    res = run_bass_kernel_spmd(_get_nc(), in_maps, core_ids=list(range(N_CORES)),
                               **(_spmd_kwargs or {}))
    ctx_un = np.zeros((B, D), dtype=np.float32)
    z = np.zeros((B, 1), dtype=np.float32)
    for c in range(N_CORES):
        ctx_un += res.results[c]["ctx_out"]
        zo = res.results[c]["z_out"]                     # [16, nchunk]
        for ci, W in enumerate(CHUNKS):
            blk = zo[0:B * W, ci].reshape(W, B)          # row = w*B + b
            z[:, 0] += blk.sum(axis=0)

    ctx = ctx_un / z                                     # [B, D] attn @ x
    context = ctx @ Wv.T                                 # [B, D]
    We = np.asarray(expert_W[eid], dtype=np.float32)     # [D, D]
    out = context @ We.T + expert_b[eid]                 # [B, D]
    resid = last + out
    mu = resid.mean(axis=-1, keepdims=True, dtype=np.float32)
    diff = resid - mu
    var = np.mean(diff * diff, axis=-1, keepdims=True, dtype=np.float32)
    new_focus = diff / np.sqrt(var + np.float32(1e-5)) * ln_gamma + ln_beta

    y = x.copy()
    y[:, -1, :] = new_focus
    return y


if __name__ == "__main__":
    rng = np.random.default_rng(0)
    xs = {
        "x_emb": rng.standard_normal((B, S, D), dtype=np.float32),
        "Wq": rng.standard_normal((D, D), dtype=np.float32) * 0.02,
        "Wk": rng.standard_normal((D, D), dtype=np.float32) * 0.02,
        "Wv": rng.standard_normal((D, D), dtype=np.float32) * 0.02,
        "expert_W": rng.standard_normal((128, D, D), dtype=np.float32) * 0.02,
        "expert_b": rng.standard_normal((128, D), dtype=np.float32) * 0.02,
        "ln_gamma": np.ones(D, dtype=np.float32),
        "ln_beta": np.zeros(D, dtype=np.float32),
        "expert_id": 7,
    }
    y = kernel(**xs)
    print(y.shape, y.dtype)
